# revision 1
# baseline (speedup 1.0000x reference)
"""Trainium2 Bass kernel for a first-order IIR low-pass filter.

y_t = alpha * x_t + (1 - alpha) * y_{t-1},  y_{-1} = 0
x: [16, 65536, 64] float32  ->  y: [16, 65536, 64] float32

Key math: beta = 1 - alpha ~= 0.718, beta^128 ~= 4e-19, so any output
timestep depends (to far below fp32 resolution) only on the previous 128
inputs.  Each 128-step output chunk is therefore an exact-enough affine
function of the current and previous 128-step input chunks:

    Y_j = A_cur @ X_j + A_prev @ X_{j-1}

with two constant 128x128 matrices, turning the serial scan into a
fully-parallel stream of PE matmuls in the natural [t, c] memory layout.

Sharding (8 cores): 4 batch-groups of 4 batches x 2 time-halves of 32768
steps.  Each core gets a 128-step halo before its time range (zeros for
the first half) so the same SPMD program runs on every core.
"""

import math
import sys

import numpy as np

try:
    import concourse.bass as bass
except ImportError:
    sys.path.insert(0, "/opt/trn_rl_repo")
    import concourse.bass as bass

import concourse.bacc as bacc
import concourse.mybir as mybir
import concourse.tile as tile
from concourse import bass_utils

SAMPLE_RATE = 16000
CUTOFF_FREQ = 1000.0
_DT = 1.0 / SAMPLE_RATE
_TAU = 1.0 / (2.0 * math.pi * CUTOFF_FREQ)
ALPHA = _DT / (_DT + _TAU)
BETA = 1.0 - ALPHA

B, T, C = 16, 65536, 64
N_CORES = 8
BG = 4              # batches per core
TH = T // 2         # timesteps per core
CHUNK = 128         # matmul chunk along time
HALO = CHUNK        # history window fed from the previous chunk
SUPER = 32          # chunks per super-chunk (DMA batching granularity)
ST = SUPER * CHUNK  # 4096 timesteps per super-chunk
NSUP = TH // ST
COLS = BG * C       # matmul moving free dim

DT_MM = mybir.dt.float32  # matmul dtype: float32 (exact) or float32r (fast)

_cached_nc = None


def _coeff_matrices():
    """lhsT weight matrices (stationary operands, already transposed).

    a_curT[k, m]  = alpha * beta^(m - k)        for k <= m else 0
    a_prevT[k, m] = alpha * beta^(m + 128 - k)
    so that (a_curT.T @ X_j + a_prevT.T @ X_{j-1})[m] = y at chunk step m.
    """
    k = np.arange(CHUNK)[:, None].astype(np.float64)
    m = np.arange(CHUNK)[None, :].astype(np.float64)
    e_cur = m - k
    a_cur = np.where(e_cur >= 0, ALPHA * BETA ** np.maximum(e_cur, 0.0), 0.0)
    a_prev = ALPHA * BETA ** (m + CHUNK - k)
    a_cur = a_cur.astype(np.float32)
    a_prev = a_prev.astype(np.float32)
    # flush near-subnormal magnitudes; they are numerically irrelevant
    a_cur[np.abs(a_cur) < 1e-30] = 0.0
    a_prev[np.abs(a_prev) < 1e-30] = 0.0
    return a_cur, a_prev


def _build_program():
    nc = bacc.Bacc("TRN2", target_bir_lowering=False, debug=False)

    x_in = nc.dram_tensor("x", [BG, HALO + TH, C], DT_MM, kind="ExternalInput").ap()
    a_cur = nc.dram_tensor("a_cur", [CHUNK, CHUNK], DT_MM, kind="ExternalInput").ap()
    a_prev = nc.dram_tensor("a_prev", [CHUNK, CHUNK], DT_MM, kind="ExternalInput").ap()
    y_out = nc.dram_tensor(
        "y", [BG, TH, C], mybir.dt.float32, kind="ExternalOutput"
    ).ap()

    with tile.TileContext(nc) as tc:
        with (
            tc.tile_pool(name="w", bufs=1) as wpool,
            tc.tile_pool(name="xin", bufs=2) as xpool,
            tc.tile_pool(name="yout", bufs=2) as ypool,
            tc.tile_pool(name="ps", bufs=8, space="PSUM") as pspool,
        ):
            wc = wpool.tile([CHUNK, CHUNK], DT_MM, tag="wc")
            wp = wpool.tile([CHUNK, CHUNK], DT_MM, tag="wp")
            nc.sync.dma_start(wc[:], a_cur[:])
            nc.sync.dma_start(wp[:], a_prev[:])

            for s in range(NSUP):
                # [partition = step-in-chunk, chunk, batch, channel];
                # chunk 0 is the 128-step halo of this super-chunk.
                xt = xpool.tile([CHUNK, SUPER + 1, BG, C], DT_MM)
                for b in range(BG):
                    src = x_in[b, s * ST : s * ST + ST + HALO, :].rearrange(
                        "(n p) c -> p n c", p=CHUNK
                    )
                    nc.sync.dma_start(xt[:, :, b, :], src)

                yt = ypool.tile([CHUNK, SUPER, BG, C], mybir.dt.float32)
                for j in range(SUPER):
                    ps = pspool.tile([CHUNK, BG, C], mybir.dt.float32)
                    nc.tensor.matmul(
                        ps[:], wp[:], xt[:, j, :, :], start=True, stop=False
                    )
                    nc.tensor.matmul(
                        ps[:], wc[:], xt[:, j + 1, :, :], start=False, stop=True
                    )
                    if j % 2 == 0:
                        nc.vector.tensor_copy(yt[:, j, :, :], ps[:])
                    else:
                        nc.scalar.copy(yt[:, j, :, :], ps[:])

                for b in range(BG):
                    dst = y_out[b, s * ST : (s + 1) * ST, :].rearrange(
                        "(n p) c -> p n c", p=CHUNK
                    )
                    nc.sync.dma_start(dst, yt[:, :, b, :])

    nc.compile()
    return nc


def _get_program():
    global _cached_nc
    if _cached_nc is None:
        _cached_nc = _build_program()
    return _cached_nc


def _shard_inputs(x):
    a_cur, a_prev = _coeff_matrices()
    in_maps = []
    for g in range(4):
        for h in range(2):
            b0 = BG * g
            t0 = TH * h
            xl = np.empty((BG, HALO + TH, C), np.float32)
            if h == 0:
                xl[:, :HALO] = 0.0
                xl[:, HALO:] = x[b0 : b0 + BG, 0:TH]
            else:
                xl[:] = x[b0 : b0 + BG, t0 - HALO : t0 + TH]
            in_maps.append({"x": xl, "a_cur": a_cur, "a_prev": a_prev})
    return in_maps


def run(x, trace=False):
    x = np.ascontiguousarray(np.asarray(x, dtype=np.float32))
    assert x.shape == (B, T, C), x.shape
    nc = _get_program()
    in_maps = _shard_inputs(x)
    res = bass_utils.run_bass_kernel_spmd(
        nc, in_maps, core_ids=list(range(N_CORES)), trace=trace
    )
    y = np.empty((B, T, C), np.float32)
    core = 0
    for g in range(4):
        for h in range(2):
            y[BG * g : BG * (g + 1), TH * h : TH * (h + 1)] = res.results[core]["y"]
            core += 1
    return y, res


def kernel(x):
    y, _ = run(x, trace=False)
    return y
